# revision 19
# baseline (speedup 1.0000x reference)
"""DeepViT re-attention block on 8 TRN2 NeuronCores.

Sharding: core c -> batch ib=c//2, query-row half ih=c%2 (512 rows).
Each core computes k/v for its full batch (1024 rows) redundantly ->
zero collectives.  kv row order is core-local (own rows first), which
is fine: attention contracts over j order-invariantly.

Per-core pipeline (matmuls fp32r or bf16, PSUM accum f32):
  A. PE-transpose w_qkv -> wT and x -> xT; qkv projections:
     qT[e,i] (own rows), kT[e,j], v[j,e] (bf16, natural layout).
  B. per i-tile(128): dots = qT.T@kT (fp32r); exp on ACT (scale=1/8,
     accum_out = softmax denom); normalize (DVE); DMA-relayout
     [i,(h,j)] -> [(i8,h16),(ig,j)]; head-mix = block-diag(w_re^T)
     matmul; LN-over-h: ones-matmul stats + partition_broadcast +
     DVE/ACT apply (in-place); PE-transpose -> [j,(i8,h)]; AV matmul.
  C. out = outT.T @ w_outT + b_out -> DRAM.
"""

import numpy as np

B, N, DIM = 4, 1024, 1024
H, DH = 16, 64
SCALE = DH ** -0.5
EPS = 1e-5
NI = 512
NJ = 1024
NCORES = 8

_CACHE = {}


def _body(nc, tc, bass, mybir):
    f32 = mybir.dt.float32
    f32r = mybir.dt.float32r
    bf16 = mybir.dt.bfloat16
    Act = mybir.ActivationFunctionType
    Alu = mybir.AluOpType
    AP = bass.AP

    xkv = nc.declare_dram_parameter("xkv", [NJ, DIM], f32, isOutput=False)
    w_qkv = nc.declare_dram_parameter("w_qkv", [3 * DIM, DIM], f32, isOutput=False)
    w_re = nc.declare_dram_parameter("w_re", [H, H], f32, isOutput=False)
    ln_g = nc.declare_dram_parameter("ln_g", [H], f32, isOutput=False)
    ln_b = nc.declare_dram_parameter("ln_b", [H], f32, isOutput=False)
    w_out = nc.declare_dram_parameter("w_out", [DIM, DIM], f32, isOutput=False)
    b_out = nc.declare_dram_parameter("b_out", [DIM], f32, isOutput=True is False)
    out = nc.declare_dram_parameter("out", [NI, DIM], f32, isOutput=True)
    import os
    DBG = bool(os.environ.get("KERNEL_DEBUG_DUMPS"))
    if DBG:
        dbg_E = nc.declare_dram_parameter("dbg_E", [128, H, NJ], f32, isOutput=True)
        dbg_Am = nc.declare_dram_parameter("dbg_Am", [128, 16, NJ], f32, isOutput=True)
        dbg_A = nc.declare_dram_parameter("dbg_A", [128, 16, NJ], f32, isOutput=True)
        dbg_q = nc.declare_dram_parameter("dbg_q", [128, NI], f32, isOutput=True)

    def cp(i, dst, src):
        # alternate copies between DVE and ACT to balance engine load
        if i % 2 == 0:
            nc.vector.tensor_copy(dst, src)
        else:
            nc.scalar.copy(dst, src)

    with tc.tile_pool(name="const", bufs=1) as const, \
         tc.tile_pool(name="big", bufs=1) as big:
        # ---------------- constants ----------------
        ident = const.tile([128, 128], f32)
        nc.gpsimd.memset(ident[:], 1.0)
        nc.gpsimd.affine_select(out=ident[:], in_=ident[:],
                                compare_op=Alu.is_ge, fill=0.0, base=0,
                                pattern=[[-1, 128]], channel_multiplier=1)
        nc.gpsimd.affine_select(out=ident[:], in_=ident[:],
                                compare_op=Alu.is_ge, fill=0.0, base=0,
                                pattern=[[1, 128]], channel_multiplier=-1)
        identb = const.tile([128, 128], bf16)
        nc.vector.tensor_copy(identb[:], ident[:])

        wret_f = const.tile([16, 16], f32)
        nc.sync.dma_start(out=wret_f[:], in_=w_re.rearrange("g h -> h g"))
        wret = const.tile([16, 16], bf16)
        nc.vector.tensor_copy(wret[:], wret_f[:])
        wblk = const.tile([128, 128], bf16)
        nc.vector.memset(wblk[:], 0.0)
        for i8 in range(8):
            nc.sync.dma_start(
                out=wblk[i8 * 16:(i8 + 1) * 16, i8 * 16:(i8 + 1) * 16],
                in_=wret[:, :])

        # Sg[(i8,g), i8'] = 1 if i8 == i8' else 0   (bf16, [128, 8])
        sg = const.tile([128, 8], bf16)
        nc.gpsimd.memset(sg[:], 1.0)
        nc.gpsimd.affine_select(out=sg[:], in_=sg[:], compare_op=Alu.is_ge,
                                fill=0.0, base=0, pattern=[[-16, 8]],
                                channel_multiplier=1)
        nc.gpsimd.affine_select(out=sg[:], in_=sg[:], compare_op=Alu.is_ge,
                                fill=0.0, base=15, pattern=[[16, 8]],
                                channel_multiplier=-1)

        # ln_g/ln_b replicated to [(i8,h), 1]
        lng_t = const.tile([128, 1], f32)
        lnb_t = const.tile([128, 1], f32)
        nc.sync.dma_start(
            out=lng_t[:, 0:1],
            in_=AP(tensor=ln_g, offset=0, ap=[[0, 8], [1, 16], [0, 1]]))
        nc.sync.dma_start(
            out=lnb_t[:, 0:1],
            in_=AP(tensor=ln_b, offset=0, ap=[[0, 8], [1, 16], [0, 1]]))

        eps_t = const.tile([128, 1], f32)
        nc.vector.memset(eps_t[:], EPS)
        bb = const.tile([128, DIM], f32)
        nc.sync.dma_start(out=bb[:],
                          in_=AP(tensor=b_out, offset=0,
                                 ap=[[0, 128], [1, DIM]]))

        # persistent activations
        qT = [big.tile([128, NI], f32r, tag=f"qT{t}", name=f"qT{t}") for t in range(8)]
        kT = [big.tile([128, NJ], f32r, tag=f"kT{t}", name=f"kT{t}") for t in range(8)]
        v = [big.tile([128, DIM], bf16, tag=f"v{t}", name=f"v{t}") for t in range(8)]
        outT = [big.tile([128, NI], f32r, tag=f"oT{t}", name=f"oT{t}") for t in range(8)]

        # ---------------- phase A: transposes + qkv ----------------
        with tc.tile_pool(name="phA", bufs=1) as phA, \
             tc.tile_pool(name="tmpA", bufs=2) as tmpA, \
             tc.tile_pool(name="psA", bufs=2, space="PSUM") as psA:
            xT = [phA.tile([128, NJ], f32r, tag=f"xT{t}", name=f"xT{t}") for t in range(8)]
            ci = 0
            for rt in range(8):
                xrow = tmpA.tile([128, DIM], f32, tag="row")
                nc.sync.dma_start(out=xrow[:],
                                  in_=xkv[rt * 128:(rt + 1) * 128, :])
                for dt in range(8):
                    pt = psA.tile([128, 128], f32, tag="ptr")
                    nc.tensor.transpose(pt[:], xrow[:, dt * 128:(dt + 1) * 128],
                                        ident[:])
                    cp(ci, xT[dt][:, rt * 128:(rt + 1) * 128], pt[:]); ci += 1

            wT = [phA.tile([128, 2048], f32r, tag=f"wT{t}", name=f"wT{t}") for t in range(8)]

            def load_wT(e0, ne):
                nonlocal ci
                for et in range(ne // 128):
                    wrow = tmpA.tile([128, DIM], f32, tag="row")
                    nc.sync.dma_start(
                        out=wrow[:],
                        in_=w_qkv[e0 + et * 128:e0 + (et + 1) * 128, :])
                    for dt in range(8):
                        pt = psA.tile([128, 128], f32, tag="ptr")
                        nc.tensor.transpose(
                            pt[:], wrow[:, dt * 128:(dt + 1) * 128], ident[:])
                        cp(ci, wT[dt][:, et * 128:(et + 1) * 128], pt[:])
                        ci += 1

            load_wT(0, 2048)          # q + k weights
            for et in range(16):
                dst, cols = (qT[et], NI) if et < 8 else (kT[et - 8], NJ)
                for rc in range(cols // 512):
                    pq = psA.tile([128, 512], f32, tag="pqkv")
                    for dt in range(8):
                        nc.tensor.matmul(
                            pq[:],
                            wT[dt][:, et * 128:(et + 1) * 128],
                            xT[dt][:, rc * 512:(rc + 1) * 512],
                            start=(dt == 0), stop=(dt == 7))
                    cp(ci, dst[:, rc * 512:(rc + 1) * 512], pq[:]); ci += 1

            load_wT(2048, 1024)       # v weights
            for rt in range(8):
                for ec in range(2):
                    pv = psA.tile([128, 512], f32, tag="pqkv")
                    for dt in range(8):
                        nc.tensor.matmul(
                            pv[:],
                            xT[dt][:, rt * 128:(rt + 1) * 128],
                            wT[dt][:, ec * 512:(ec + 1) * 512],
                            start=(dt == 0), stop=(dt == 7))
                    cp(ci, v[rt][:, ec * 512:(ec + 1) * 512], pv[:]); ci += 1

        if DBG:
            nc.sync.dma_start(out=dbg_q[:, :], in_=qT[0][:, :].bitcast(f32))
        # ---------------- phase B: attention ----------------
        with tc.tile_pool(name="phB", bufs=1) as phB, \
             tc.tile_pool(name="attw", bufs=1) as attw, \
             tc.tile_pool(name="psD", bufs=2, space="PSUM") as psD, \
             tc.tile_pool(name="psAV", bufs=2, space="PSUM") as psAV, \
             tc.tile_pool(name="psM", bufs=1, space="PSUM") as psM:
            for it in range(4):
                its = slice(it * 128, (it + 1) * 128)
                E = phB.tile([128, H, NJ], bf16, tag="E")
                rs = phB.tile([128, H, 2], f32, tag="rs")
                rcp = phB.tile([128, H], f32, tag="rcp")
                for h in range(16):
                    et, po = h // 2, (h % 2) * 64
                    for jc in range(2):
                        js = slice(jc * 512, (jc + 1) * 512)
                        pd = psD.tile([128, 512], f32, tag="pdots")
                        nc.tensor.matmul(
                            pd[:],
                            qT[et][po:po + 64, its],
                            kT[et][po:po + 64, js],
                            start=True, stop=True)
                        nc.scalar.activation(
                            out=E[:, h, js], in_=pd[:],
                            func=Act.Exp, scale=SCALE,
                            accum_out=rs[:, h, jc:jc + 1])
                    nc.vector.tensor_add(rs[:, h, 0:1], rs[:, h, 0:1],
                                         rs[:, h, 1:2])
                nc.vector.reciprocal(rcp[:], rs[:, :, 0])
                for h in range(16):
                    nc.vector.tensor_scalar_mul(E[:, h, :], E[:, h, :],
                                                rcp[:, h:h + 1])

                if DBG and it == 0:
                    nc.gpsimd.dma_start(out=dbg_E[:, :, :], in_=E[:])
                # relayout: A[(i8,h), ig, j] <- E[ig*8+i8, h, j]
                A = phB.tile([128, 16, NJ], bf16, tag="A")
                for ig in range(16):
                    nc.sync.dma_start(
                        out=A[:, ig, :],
                        in_=E[ig * 8:(ig + 1) * 8, :, :])

                # head mix + LN (in-place into A)
                for ig in range(16):
                    for jc in range(2):
                        js = slice(jc * 512, (jc + 1) * 512)
                        pm = psM.tile([128, 512], f32, tag="pmix")
                        nc.tensor.matmul(pm[:], wblk[:], A[:, ig, js],
                                         start=True, stop=True)
                        M = phB.tile([128, 512], bf16, tag="M")
                        nc.vector.tensor_copy(M[:], pm[:])
                        if DBG and it == 0:
                            nc.gpsimd.dma_start(out=dbg_Am[:, ig, js], in_=M[:])
                        M2 = phB.tile([128, 512], bf16, tag="M2")
                        nc.vector.tensor_mul(M2[:], M[:], M[:])
                        s1 = psM.tile([8, 512], f32, tag="stat1")
                        nc.tensor.matmul(s1[:], sg[:], M[:],
                                         start=True, stop=True)
                        s2 = psM.tile([8, 512], f32, tag="stat2")
                        nc.tensor.matmul(s2[:], sg[:], M2[:],
                                         start=True, stop=True)
                        mu = phB.tile([8, 512], f32, tag="mu")
                        nc.scalar.mul(mu[:], s1[:], 1.0 / 16.0)
                        mu2 = phB.tile([8, 512], f32, tag="mu2")
                        nc.vector.tensor_mul(mu2[:], mu[:], mu[:])
                        var = phB.tile([8, 512], f32, tag="var")
                        nc.scalar.mul(var[:], s2[:], 1.0 / 16.0)
                        nc.vector.tensor_sub(var[:], var[:], mu2[:])
                        rstd = phB.tile([8, 512], f32, tag="rstd")
                        nc.scalar.activation(out=rstd[:], in_=var[:],
                                             func=Act.Sqrt,
                                             bias=eps_t[0:8, 0:1], scale=1.0)
                        nc.vector.reciprocal(rstd[:], rstd[:])
                        mub = phB.tile([128, 512], f32, tag="mub")
                        rstdb = phB.tile([128, 512], f32, tag="rstdb")
                        for dst, src_t in ((mub, mu), (rstdb, rstd)):
                            sap = src_t[:, :]
                            nc.sync.dma_start(
                                out=dst[:, :],
                                in_=AP(tensor=sap.tensor, offset=sap.offset,
                                       ap=[sap.ap[0], [0, 16], sap.ap[1]]))
                        nc.vector.tensor_sub(M[:], M[:], mub[:])
                        nc.vector.tensor_mul(M[:], M[:], rstdb[:])
                        nc.scalar.activation(out=A[:, ig, js], in_=M[:],
                                             func=Act.Identity,
                                             bias=lnb_t[:, 0:1],
                                             scale=lng_t[:, 0:1])

                if DBG and it == 0:
                    nc.gpsimd.dma_start(out=dbg_A[:, :, :], in_=A[:])
                # AV: transpose all A blocks first, then per-head
                # sequential PSUM chains (one start/stop pair at a time
                # per bank region), copying each head-pair out before the
                # next chain re-marks the zero region.
                atts = []
                ci2 = 0
                for jt in range(8):
                    att = attw.tile([128, 16, 8, 16], bf16, tag=f"att{jt}",
                                    name=f"att{jt}")
                    atts.append(att)
                    for ig in range(16):
                        pt = psM.tile([128, 128], bf16, tag="ptb")
                        nc.tensor.transpose(
                            pt[:], A[:, ig, jt * 128:(jt + 1) * 128], identb[:])
                        cp(ci2, att[:, ig, :, :].rearrange("p a b -> p (a b)"),
                           pt[:])
                        ci2 += 1
                for et in range(8):
                    av = psAV.tile([128, 128], f32, tag="av", name="av")
                    for hh in range(2):
                        h = 2 * et + hh
                        for jt in range(8):
                            nc.tensor.matmul(
                                av[hh * 64:(hh + 1) * 64, :],
                                v[jt][:, h * 64:(h + 1) * 64],
                                atts[jt][:, :, :, h],
                                start=(jt == 0), stop=(jt == 7),
                                skip_group_check=True)
                    cp(et, outT[et][:, its], av[:, :])

        # ---------------- phase C: output projection ----------------
        with tc.tile_pool(name="phC", bufs=1) as phC, \
             tc.tile_pool(name="tmpC", bufs=2) as tmpC, \
             tc.tile_pool(name="psC", bufs=2, space="PSUM") as psC:
            woT = [phC.tile([128, DIM], f32r, tag=f"woT{t}", name=f"woT{t}") for t in range(8)]
            ci = 0
            for et in range(8):
                wrow = tmpC.tile([128, DIM], f32, tag="row")
                nc.sync.dma_start(out=wrow[:],
                                  in_=w_out[et * 128:(et + 1) * 128, :])
                for dt in range(8):
                    pt = psC.tile([128, 128], f32, tag="ptr")
                    nc.tensor.transpose(pt[:], wrow[:, dt * 128:(dt + 1) * 128],
                                        ident[:])
                    cp(ci, woT[dt][:, et * 128:(et + 1) * 128], pt[:]); ci += 1
            for it in range(4):
                for mc in range(2):
                    pf = psC.tile([128, 512], f32, tag="pfin")
                    for et in range(8):
                        nc.tensor.matmul(
                            pf[:],
                            outT[et][:, it * 128:(it + 1) * 128],
                            woT[et][:, mc * 512:(mc + 1) * 512],
                            start=(et == 0), stop=(et == 7))
                    ob = tmpC.tile([128, 512], f32, tag="ob")
                    nc.vector.tensor_add(ob[:], pf[:],
                                         bb[:, mc * 512:(mc + 1) * 512])
                    nc.sync.dma_start(
                        out=out[it * 128:(it + 1) * 128,
                                mc * 512:(mc + 1) * 512],
                        in_=ob[:])


def _get_nc():
    if "nc" not in _CACHE:
        import concourse.bass as bass
        import concourse.mybir as mybir
        import concourse.tile as tile
        from concourse import bacc
        nc = bacc.Bacc("TRN2", target_bir_lowering=False, debug=False, num_devices=NCORES)
        with tile.TileContext(nc) as tc:
            _body(nc, tc, bass, mybir)
        nc.finalize()
        _CACHE["nc"] = nc
    return _CACHE["nc"]


def kernel(x, w_qkv, w_re, ln_g, ln_b, w_out, b_out, _trace=False):
    from concourse.bass_utils import run_bass_kernel_spmd
    nc = _get_nc()
    x = np.asarray(x, dtype=np.float32)
    in_maps = []
    for c in range(NCORES):
        ib, ih = c // 2, c % 2
        own = x[ib, ih * NI:(ih + 1) * NI]
        oth = x[ib, (1 - ih) * NI:(2 - ih) * NI]
        in_maps.append({
            "xkv": np.ascontiguousarray(np.concatenate([own, oth], axis=0)),
            "w_qkv": np.asarray(w_qkv, np.float32),
            "w_re": np.asarray(w_re, np.float32),
            "ln_g": np.asarray(ln_g, np.float32),
            "ln_b": np.asarray(ln_b, np.float32),
            "w_out": np.asarray(w_out, np.float32),
            "b_out": np.asarray(b_out, np.float32),
        })
    res = run_bass_kernel_spmd(nc, in_maps, list(range(NCORES)), trace=_trace)
    outp = np.empty((B, N, DIM), np.float32)
    for c in range(NCORES):
        ib, ih = c // 2, c % 2
        outp[ib, ih * NI:(ih + 1) * NI] = res.results[c]["out"]
    if _trace:
        return outp, res
    return outp


# revision 20
# speedup vs baseline: 1.0130x; 1.0130x over previous
"""DeepViT re-attention block on 8 TRN2 NeuronCores.

Sharding: core c -> batch ib=c//2, query-row half ih=c%2 (512 rows).
Each core computes k/v for its full batch (1024 rows) redundantly ->
zero collectives.  kv row order is core-local (own rows first), which
is fine: attention contracts over j order-invariantly.

Per-core pipeline (matmuls fp32r or bf16, PSUM accum f32):
  A. PE-transpose w_qkv -> wT and x -> xT; qkv projections:
     qT[e,i] (own rows), kT[e,j], v[j,e] (bf16, natural layout).
  B. per i-tile(128): dots = qT.T@kT (fp32r); exp on ACT (scale=1/8,
     accum_out = softmax denom); normalize (DVE); DMA-relayout
     [i,(h,j)] -> [(i8,h16),(ig,j)]; head-mix = block-diag(w_re^T)
     matmul; LN-over-h: ones-matmul stats + partition_broadcast +
     DVE/ACT apply (in-place); PE-transpose -> [j,(i8,h)]; AV matmul.
  C. out = outT.T @ w_outT + b_out -> DRAM.
"""

import numpy as np

B, N, DIM = 4, 1024, 1024
H, DH = 16, 64
SCALE = DH ** -0.5
EPS = 1e-5
NI = 512
NJ = 1024
NCORES = 8

_CACHE = {}


def _body(nc, tc, bass, mybir):
    f32 = mybir.dt.float32
    f32r = mybir.dt.float32r
    bf16 = mybir.dt.bfloat16
    Act = mybir.ActivationFunctionType
    Alu = mybir.AluOpType
    AP = bass.AP

    xkv = nc.declare_dram_parameter("xkv", [NJ, DIM], f32, isOutput=False)
    w_qkv = nc.declare_dram_parameter("w_qkv", [3 * DIM, DIM], f32, isOutput=False)
    w_re = nc.declare_dram_parameter("w_re", [H, H], f32, isOutput=False)
    ln_g = nc.declare_dram_parameter("ln_g", [H], f32, isOutput=False)
    ln_b = nc.declare_dram_parameter("ln_b", [H], f32, isOutput=False)
    w_out = nc.declare_dram_parameter("w_out", [DIM, DIM], f32, isOutput=False)
    b_out = nc.declare_dram_parameter("b_out", [DIM], f32, isOutput=False)
    out = nc.declare_dram_parameter("out", [NI, DIM], f32, isOutput=True)
    import os
    DBG = bool(os.environ.get("KERNEL_DEBUG_DUMPS"))
    if DBG:
        dbg_E = nc.declare_dram_parameter("dbg_E", [128, H, NJ], f32, isOutput=True)
        dbg_Am = nc.declare_dram_parameter("dbg_Am", [128, 16, NJ], f32, isOutput=True)
        dbg_A = nc.declare_dram_parameter("dbg_A", [128, 16, NJ], f32, isOutput=True)
        dbg_q = nc.declare_dram_parameter("dbg_q", [128, NI], f32, isOutput=True)

    def cp(i, dst, src):
        # alternate copies between DVE and ACT to balance engine load
        if i % 2 == 0:
            nc.vector.tensor_copy(dst, src)
        else:
            nc.scalar.copy(dst, src)

    with tc.tile_pool(name="const", bufs=1) as const, \
         tc.tile_pool(name="big", bufs=1) as big:
        # ---------------- constants ----------------
        ident = const.tile([128, 128], f32)
        nc.gpsimd.memset(ident[:], 1.0)
        nc.gpsimd.affine_select(out=ident[:], in_=ident[:],
                                compare_op=Alu.is_ge, fill=0.0, base=0,
                                pattern=[[-1, 128]], channel_multiplier=1)
        nc.gpsimd.affine_select(out=ident[:], in_=ident[:],
                                compare_op=Alu.is_ge, fill=0.0, base=0,
                                pattern=[[1, 128]], channel_multiplier=-1)
        identb = const.tile([128, 128], bf16)
        nc.vector.tensor_copy(identb[:], ident[:])

        wret_f = const.tile([16, 16], f32)
        nc.sync.dma_start(out=wret_f[:], in_=w_re.rearrange("g h -> h g"))
        wret = const.tile([16, 16], bf16)
        nc.vector.tensor_copy(wret[:], wret_f[:])
        wblk = const.tile([128, 128], bf16)
        nc.vector.memset(wblk[:], 0.0)
        for i8 in range(8):
            nc.sync.dma_start(
                out=wblk[i8 * 16:(i8 + 1) * 16, i8 * 16:(i8 + 1) * 16],
                in_=wret[:, :])

        # Sg[(i8,g), i8'] = 1 if i8 == i8' else 0   (bf16, [128, 8])
        sg = const.tile([128, 8], bf16)
        nc.gpsimd.memset(sg[:], 1.0)
        nc.gpsimd.affine_select(out=sg[:], in_=sg[:], compare_op=Alu.is_ge,
                                fill=0.0, base=0, pattern=[[-16, 8]],
                                channel_multiplier=1)
        nc.gpsimd.affine_select(out=sg[:], in_=sg[:], compare_op=Alu.is_ge,
                                fill=0.0, base=15, pattern=[[16, 8]],
                                channel_multiplier=-1)

        # ln_g/ln_b replicated to [(i8,h), 1]
        lng_t = const.tile([128, 1], f32)
        lnb_t = const.tile([128, 1], f32)
        nc.sync.dma_start(
            out=lng_t[:, 0:1],
            in_=AP(tensor=ln_g, offset=0, ap=[[0, 8], [1, 16], [0, 1]]))
        nc.sync.dma_start(
            out=lnb_t[:, 0:1],
            in_=AP(tensor=ln_b, offset=0, ap=[[0, 8], [1, 16], [0, 1]]))

        eps_t = const.tile([128, 1], f32)
        nc.vector.memset(eps_t[:], EPS)
        bb = const.tile([128, DIM], f32)
        nc.sync.dma_start(out=bb[:],
                          in_=AP(tensor=b_out, offset=0,
                                 ap=[[0, 128], [1, DIM]]))

        # persistent activations
        qT = [big.tile([128, NI], f32r, tag=f"qT{t}", name=f"qT{t}") for t in range(8)]
        kT = [big.tile([128, NJ], f32r, tag=f"kT{t}", name=f"kT{t}") for t in range(8)]
        v = [big.tile([128, DIM], bf16, tag=f"v{t}", name=f"v{t}") for t in range(8)]
        outT = [big.tile([128, NI], f32r, tag=f"oT{t}", name=f"oT{t}") for t in range(8)]

        # ---------------- phase A: transposes + qkv ----------------
        with tc.tile_pool(name="phA", bufs=1) as phA, \
             tc.tile_pool(name="tmpA", bufs=2) as tmpA, \
             tc.tile_pool(name="psA", bufs=2, space="PSUM") as psA:
            xT = [phA.tile([128, NJ], f32r, tag=f"xT{t}", name=f"xT{t}") for t in range(8)]
            ci = 0
            for rt in range(8):
                xrow = tmpA.tile([128, DIM], f32, tag="row")
                nc.sync.dma_start(out=xrow[:],
                                  in_=xkv[rt * 128:(rt + 1) * 128, :])
                for dt in range(8):
                    pt = psA.tile([128, 128], f32, tag="ptr")
                    nc.tensor.transpose(pt[:], xrow[:, dt * 128:(dt + 1) * 128],
                                        ident[:])
                    cp(ci, xT[dt][:, rt * 128:(rt + 1) * 128], pt[:]); ci += 1

            wT = [phA.tile([128, 2048], f32r, tag=f"wT{t}", name=f"wT{t}") for t in range(8)]

            def load_wT(e0, ne):
                nonlocal ci
                for et in range(ne // 128):
                    wrow = tmpA.tile([128, DIM], f32, tag="row")
                    nc.sync.dma_start(
                        out=wrow[:],
                        in_=w_qkv[e0 + et * 128:e0 + (et + 1) * 128, :])
                    for dt in range(8):
                        pt = psA.tile([128, 128], f32, tag="ptr")
                        nc.tensor.transpose(
                            pt[:], wrow[:, dt * 128:(dt + 1) * 128], ident[:])
                        cp(ci, wT[dt][:, et * 128:(et + 1) * 128], pt[:])
                        ci += 1

            load_wT(0, 2048)          # q + k weights
            for et in range(16):
                dst, cols = (qT[et], NI) if et < 8 else (kT[et - 8], NJ)
                for rc in range(cols // 512):
                    pq = psA.tile([128, 512], f32, tag="pqkv")
                    for dt in range(8):
                        nc.tensor.matmul(
                            pq[:],
                            wT[dt][:, et * 128:(et + 1) * 128],
                            xT[dt][:, rc * 512:(rc + 1) * 512],
                            start=(dt == 0), stop=(dt == 7))
                    cp(ci, dst[:, rc * 512:(rc + 1) * 512], pq[:]); ci += 1

            load_wT(2048, 1024)       # v weights
            for rt in range(8):
                for ec in range(2):
                    pv = psA.tile([128, 512], f32, tag="pqkv")
                    for dt in range(8):
                        nc.tensor.matmul(
                            pv[:],
                            xT[dt][:, rt * 128:(rt + 1) * 128],
                            wT[dt][:, ec * 512:(ec + 1) * 512],
                            start=(dt == 0), stop=(dt == 7))
                    cp(ci, v[rt][:, ec * 512:(ec + 1) * 512], pv[:]); ci += 1

        if DBG:
            nc.sync.dma_start(out=dbg_q[:, :], in_=qT[0][:, :].bitcast(f32))
        # ---------------- phase B: attention ----------------
        with tc.tile_pool(name="phB", bufs=1) as phB, \
             tc.tile_pool(name="attw", bufs=1) as attw, \
             tc.tile_pool(name="psD", bufs=2, space="PSUM") as psD, \
             tc.tile_pool(name="psAV", bufs=2, space="PSUM") as psAV, \
             tc.tile_pool(name="psM", bufs=1, space="PSUM") as psM:
            for it in range(4):
                its = slice(it * 128, (it + 1) * 128)
                E = phB.tile([128, H, NJ], bf16, tag="E")
                rs = phB.tile([128, H, 2], f32, tag="rs")
                rcp = phB.tile([128, H], f32, tag="rcp")
                for h in range(16):
                    et, po = h // 2, (h % 2) * 64
                    for jc in range(2):
                        js = slice(jc * 512, (jc + 1) * 512)
                        pd = psD.tile([128, 512], f32, tag="pdots")
                        nc.tensor.matmul(
                            pd[:],
                            qT[et][po:po + 64, its],
                            kT[et][po:po + 64, js],
                            start=True, stop=True)
                        nc.scalar.activation(
                            out=E[:, h, js], in_=pd[:],
                            func=Act.Exp, scale=SCALE,
                            accum_out=rs[:, h, jc:jc + 1])
                    nc.vector.tensor_add(rs[:, h, 0:1], rs[:, h, 0:1],
                                         rs[:, h, 1:2])
                nc.vector.reciprocal(rcp[:], rs[:, :, 0])
                for h in range(16):
                    nc.vector.tensor_scalar_mul(E[:, h, :], E[:, h, :],
                                                rcp[:, h:h + 1])

                if DBG and it == 0:
                    nc.gpsimd.dma_start(out=dbg_E[:, :, :], in_=E[:])
                # relayout: A[(i8,h), ig, j] <- E[ig*8+i8, h, j]
                A = phB.tile([128, 16, NJ], bf16, tag="A")
                for ig in range(16):
                    nc.sync.dma_start(
                        out=A[:, ig, :],
                        in_=E[ig * 8:(ig + 1) * 8, :, :])

                # head mix + LN (in-place into A)
                for ig in range(16):
                    for jc in range(2):
                        js = slice(jc * 512, (jc + 1) * 512)
                        pm = psM.tile([128, 512], f32, tag="pmix")
                        nc.tensor.matmul(pm[:], wblk[:], A[:, ig, js],
                                         start=True, stop=True)
                        M = phB.tile([128, 512], bf16, tag="M")
                        nc.vector.tensor_copy(M[:], pm[:])
                        if DBG and it == 0:
                            nc.gpsimd.dma_start(out=dbg_Am[:, ig, js], in_=M[:])
                        M2 = phB.tile([128, 512], bf16, tag="M2")
                        nc.vector.tensor_mul(M2[:], M[:], M[:])
                        s1 = psM.tile([8, 512], f32, tag="stat1")
                        nc.tensor.matmul(s1[:], sg[:], M[:],
                                         start=True, stop=True)
                        s2 = psM.tile([8, 512], f32, tag="stat2")
                        nc.tensor.matmul(s2[:], sg[:], M2[:],
                                         start=True, stop=True)
                        mu = phB.tile([8, 512], f32, tag="mu")
                        nc.scalar.mul(mu[:], s1[:], 1.0 / 16.0)
                        mu2 = phB.tile([8, 512], f32, tag="mu2")
                        nc.vector.tensor_mul(mu2[:], mu[:], mu[:])
                        var = phB.tile([8, 512], f32, tag="var")
                        nc.scalar.mul(var[:], s2[:], 1.0 / 16.0)
                        nc.vector.tensor_sub(var[:], var[:], mu2[:])
                        rstd = phB.tile([8, 512], f32, tag="rstd")
                        nc.scalar.activation(out=rstd[:], in_=var[:],
                                             func=Act.Sqrt,
                                             bias=eps_t[0:8, 0:1], scale=1.0)
                        nc.vector.reciprocal(rstd[:], rstd[:])
                        mub = phB.tile([128, 512], f32, tag="mub")
                        rstdb = phB.tile([128, 512], f32, tag="rstdb")
                        for dst, src_t in ((mub, mu), (rstdb, rstd)):
                            sap = src_t[:, :]
                            nc.sync.dma_start(
                                out=dst[:, :],
                                in_=AP(tensor=sap.tensor, offset=sap.offset,
                                       ap=[sap.ap[0], [0, 16], sap.ap[1]]))
                        nc.vector.tensor_sub(M[:], M[:], mub[:])
                        nc.vector.tensor_mul(M[:], M[:], rstdb[:])
                        nc.scalar.activation(out=A[:, ig, js], in_=M[:],
                                             func=Act.Identity,
                                             bias=lnb_t[:, 0:1],
                                             scale=lng_t[:, 0:1])

                if DBG and it == 0:
                    nc.gpsimd.dma_start(out=dbg_A[:, :, :], in_=A[:])
                # AV: transpose all A blocks first, then per-head
                # sequential PSUM chains (one start/stop pair at a time
                # per bank region), copying each head-pair out before the
                # next chain re-marks the zero region.
                atts = []
                ci2 = 0
                for jt in range(8):
                    att = attw.tile([128, 16, 8, 16], bf16, tag=f"att{jt}",
                                    name=f"att{jt}")
                    atts.append(att)
                    for ig in range(16):
                        pt = psM.tile([128, 128], bf16, tag="ptb")
                        nc.tensor.transpose(
                            pt[:], A[:, ig, jt * 128:(jt + 1) * 128], identb[:])
                        cp(ci2, att[:, ig, :, :].rearrange("p a b -> p (a b)"),
                           pt[:])
                        ci2 += 1
                for et in range(8):
                    av = psAV.tile([128, 128], f32, tag="av", name="av")
                    for hh in range(2):
                        h = 2 * et + hh
                        for jt in range(8):
                            nc.tensor.matmul(
                                av[hh * 64:(hh + 1) * 64, :],
                                v[jt][:, h * 64:(h + 1) * 64],
                                atts[jt][:, :, :, h],
                                start=(jt == 0), stop=(jt == 7),
                                skip_group_check=True)
                    cp(et, outT[et][:, its], av[:, :])

        # ---------------- phase C: output projection ----------------
        with tc.tile_pool(name="phC", bufs=1) as phC, \
             tc.tile_pool(name="tmpC", bufs=2) as tmpC, \
             tc.tile_pool(name="psC", bufs=2, space="PSUM") as psC:
            woT = [phC.tile([128, DIM], f32r, tag=f"woT{t}", name=f"woT{t}") for t in range(8)]
            ci = 0
            for et in range(8):
                wrow = tmpC.tile([128, DIM], f32, tag="row")
                nc.sync.dma_start(out=wrow[:],
                                  in_=w_out[et * 128:(et + 1) * 128, :])
                for dt in range(8):
                    pt = psC.tile([128, 128], f32, tag="ptr")
                    nc.tensor.transpose(pt[:], wrow[:, dt * 128:(dt + 1) * 128],
                                        ident[:])
                    cp(ci, woT[dt][:, et * 128:(et + 1) * 128], pt[:]); ci += 1
            for it in range(4):
                for mc in range(2):
                    pf = psC.tile([128, 512], f32, tag="pfin")
                    for et in range(8):
                        nc.tensor.matmul(
                            pf[:],
                            outT[et][:, it * 128:(it + 1) * 128],
                            woT[et][:, mc * 512:(mc + 1) * 512],
                            start=(et == 0), stop=(et == 7))
                    ob = tmpC.tile([128, 512], f32, tag="ob")
                    nc.vector.tensor_add(ob[:], pf[:],
                                         bb[:, mc * 512:(mc + 1) * 512])
                    nc.sync.dma_start(
                        out=out[it * 128:(it + 1) * 128,
                                mc * 512:(mc + 1) * 512],
                        in_=ob[:])


def _get_nc():
    if "nc" not in _CACHE:
        import concourse.bass as bass
        import concourse.mybir as mybir
        import concourse.tile as tile
        from concourse import bacc
        nc = bacc.Bacc("TRN2", target_bir_lowering=False, debug=False, num_devices=NCORES)
        with tile.TileContext(nc) as tc:
            _body(nc, tc, bass, mybir)
        nc.finalize()
        _CACHE["nc"] = nc
    return _CACHE["nc"]


def kernel(x, w_qkv, w_re, ln_g, ln_b, w_out, b_out, _trace=False):
    from concourse.bass_utils import run_bass_kernel_spmd
    nc = _get_nc()
    x = np.asarray(x, dtype=np.float32)
    in_maps = []
    for c in range(NCORES):
        ib, ih = c // 2, c % 2
        own = x[ib, ih * NI:(ih + 1) * NI]
        oth = x[ib, (1 - ih) * NI:(2 - ih) * NI]
        in_maps.append({
            "xkv": np.ascontiguousarray(np.concatenate([own, oth], axis=0)),
            "w_qkv": np.asarray(w_qkv, np.float32),
            "w_re": np.asarray(w_re, np.float32),
            "ln_g": np.asarray(ln_g, np.float32),
            "ln_b": np.asarray(ln_b, np.float32),
            "w_out": np.asarray(w_out, np.float32),
            "b_out": np.asarray(b_out, np.float32),
        })
    res = run_bass_kernel_spmd(nc, in_maps, list(range(NCORES)), trace=_trace)
    outp = np.empty((B, N, DIM), np.float32)
    for c in range(NCORES):
        ib, ih = c // 2, c % 2
        outp[ib, ih * NI:(ih + 1) * NI] = res.results[c]["out"]
    if _trace:
        return outp, res
    return outp


# revision 21
# speedup vs baseline: 1.0982x; 1.0841x over previous
"""DeepViT re-attention block on 8 TRN2 NeuronCores.

Sharding: core c -> batch ib=c//2, query-row half ih=c%2 (512 rows).
Each core computes k/v for its full batch (1024 rows) redundantly ->
zero collectives.  kv row order is core-local (own rows first), which
is fine: attention contracts over j order-invariantly.

Per-core pipeline (matmuls fp32r or bf16, PSUM accum f32):
  A. PE-transpose w_qkv -> wT and x -> xT; qkv projections:
     qT[e,i] (own rows), kT[e,j], v[j,e] (bf16, natural layout).
  B. per i-tile(128): dots = qT.T@kT (fp32r); exp on ACT (scale=1/8,
     accum_out = softmax denom); normalize (DVE); DMA-relayout
     [i,(h,j)] -> [(i8,h16),(ig,j)]; head-mix = block-diag(w_re^T)
     matmul; LN-over-h: ones-matmul stats + partition_broadcast +
     DVE/ACT apply (in-place); PE-transpose -> [j,(i8,h)]; AV matmul.
  C. out = outT.T @ w_outT + b_out -> DRAM.
"""

import numpy as np

B, N, DIM = 4, 1024, 1024
H, DH = 16, 64
SCALE = DH ** -0.5
EPS = 1e-5
NI = 512
NJ = 1024
NCORES = 8

_CACHE = {}


def _body(nc, tc, bass, mybir):
    f32 = mybir.dt.float32
    f32r = mybir.dt.float32r
    bf16 = mybir.dt.bfloat16
    Act = mybir.ActivationFunctionType
    Alu = mybir.AluOpType
    AP = bass.AP

    xkv = nc.declare_dram_parameter("xkv", [NJ, DIM], f32, isOutput=False)
    w_qkv = nc.declare_dram_parameter("w_qkv", [3 * DIM, DIM], f32, isOutput=False)
    w_re = nc.declare_dram_parameter("w_re", [H, H], f32, isOutput=False)
    ln_g = nc.declare_dram_parameter("ln_g", [H], f32, isOutput=False)
    ln_b = nc.declare_dram_parameter("ln_b", [H], f32, isOutput=False)
    w_out = nc.declare_dram_parameter("w_out", [DIM, DIM], f32, isOutput=False)
    b_out = nc.declare_dram_parameter("b_out", [DIM], f32, isOutput=False)
    out = nc.declare_dram_parameter("out", [NI, DIM], f32, isOutput=True)
    import os
    DBG = bool(os.environ.get("KERNEL_DEBUG_DUMPS"))
    if DBG:
        dbg_E = nc.declare_dram_parameter("dbg_E", [128, H, NJ], f32, isOutput=True)
        dbg_Am = nc.declare_dram_parameter("dbg_Am", [128, 16, NJ], f32, isOutput=True)
        dbg_A = nc.declare_dram_parameter("dbg_A", [128, 16, NJ], f32, isOutput=True)
        dbg_q = nc.declare_dram_parameter("dbg_q", [128, NI], f32, isOutput=True)

    def cp(i, dst, src):
        # alternate copies between DVE and ACT to balance engine load
        if i % 2 == 0:
            nc.vector.tensor_copy(dst, src)
        else:
            nc.scalar.copy(dst, src)

    with tc.tile_pool(name="const", bufs=1) as const, \
         tc.tile_pool(name="big", bufs=1) as big:
        # ---------------- constants ----------------
        ident = const.tile([128, 128], f32)
        nc.gpsimd.memset(ident[:], 1.0)
        nc.gpsimd.affine_select(out=ident[:], in_=ident[:],
                                compare_op=Alu.is_ge, fill=0.0, base=0,
                                pattern=[[-1, 128]], channel_multiplier=1)
        nc.gpsimd.affine_select(out=ident[:], in_=ident[:],
                                compare_op=Alu.is_ge, fill=0.0, base=0,
                                pattern=[[1, 128]], channel_multiplier=-1)
        identb = const.tile([128, 128], bf16)
        nc.vector.tensor_copy(identb[:], ident[:])

        wret_f = const.tile([16, 16], f32)
        nc.sync.dma_start(out=wret_f[:], in_=w_re.rearrange("g h -> h g"))
        wret = const.tile([16, 16], bf16)
        nc.vector.tensor_copy(wret[:], wret_f[:])
        wblk = const.tile([128, 128], bf16)
        nc.vector.memset(wblk[:], 0.0)
        for i8 in range(8):
            nc.sync.dma_start(
                out=wblk[i8 * 16:(i8 + 1) * 16, i8 * 16:(i8 + 1) * 16],
                in_=wret[:, :])

        # Sg[(i8,g), i8'] = 1 if i8 == i8' else 0   (bf16, [128, 8])
        sg = const.tile([128, 8], bf16)
        nc.gpsimd.memset(sg[:], 1.0)
        nc.gpsimd.affine_select(out=sg[:], in_=sg[:], compare_op=Alu.is_ge,
                                fill=0.0, base=0, pattern=[[-16, 8]],
                                channel_multiplier=1)
        nc.gpsimd.affine_select(out=sg[:], in_=sg[:], compare_op=Alu.is_ge,
                                fill=0.0, base=15, pattern=[[16, 8]],
                                channel_multiplier=-1)

        # ln_g/ln_b replicated to [(i8,h), 1]
        lng_t = const.tile([128, 1], f32)
        lnb_t = const.tile([128, 1], f32)
        nc.sync.dma_start(
            out=lng_t[:, 0:1],
            in_=AP(tensor=ln_g, offset=0, ap=[[0, 8], [1, 16], [0, 1]]))
        nc.sync.dma_start(
            out=lnb_t[:, 0:1],
            in_=AP(tensor=ln_b, offset=0, ap=[[0, 8], [1, 16], [0, 1]]))

        eps_t = const.tile([128, 1], f32)
        nc.vector.memset(eps_t[:], EPS)
        bb = const.tile([128, DIM], f32)
        nc.sync.dma_start(out=bb[:],
                          in_=AP(tensor=b_out, offset=0,
                                 ap=[[0, 128], [1, DIM]]))

        # persistent activations
        qT = [big.tile([128, NI], f32r, tag=f"qT{t}", name=f"qT{t}") for t in range(8)]
        kT = [big.tile([128, NJ], f32r, tag=f"kT{t}", name=f"kT{t}") for t in range(8)]
        v = [big.tile([128, DIM], bf16, tag=f"v{t}", name=f"v{t}") for t in range(8)]
        outT = [big.tile([128, NI], f32r, tag=f"oT{t}", name=f"oT{t}") for t in range(8)]

        # ---------------- phase A: transposes + qkv ----------------
        with tc.tile_pool(name="phA", bufs=1) as phA, \
             tc.tile_pool(name="tmpA", bufs=2) as tmpA, \
             tc.tile_pool(name="psA", bufs=3, space="PSUM") as psA:
            xT = [phA.tile([128, NJ], f32r, tag=f"xT{t}", name=f"xT{t}") for t in range(8)]
            ci = 0
            for rt in range(8):
                xrow = tmpA.tile([128, DIM], f32, tag="row")
                nc.sync.dma_start(out=xrow[:],
                                  in_=xkv[rt * 128:(rt + 1) * 128, :])
                for dt in range(8):
                    pt = psA.tile([128, 128], f32, tag="ptr")
                    nc.tensor.transpose(pt[:], xrow[:, dt * 128:(dt + 1) * 128],
                                        ident[:])
                    cp(ci, xT[dt][:, rt * 128:(rt + 1) * 128], pt[:]); ci += 1

            wT = [phA.tile([128, 2048], f32r, tag=f"wT{t}", name=f"wT{t}") for t in range(8)]

            def load_wT(e0, ne):
                nonlocal ci
                for et in range(ne // 128):
                    wrow = tmpA.tile([128, DIM], f32, tag="row")
                    nc.sync.dma_start(
                        out=wrow[:],
                        in_=w_qkv[e0 + et * 128:e0 + (et + 1) * 128, :])
                    for dt in range(8):
                        pt = psA.tile([128, 128], f32, tag="ptr")
                        nc.tensor.transpose(
                            pt[:], wrow[:, dt * 128:(dt + 1) * 128], ident[:])
                        cp(ci, wT[dt][:, et * 128:(et + 1) * 128], pt[:])
                        ci += 1

            load_wT(0, 2048)          # q + k weights
            for et in range(16):
                dst, cols = (qT[et], NI) if et < 8 else (kT[et - 8], NJ)
                for rc in range(cols // 512):
                    pq = psA.tile([128, 512], f32, tag="pqkv")
                    for dt in range(8):
                        nc.tensor.matmul(
                            pq[:],
                            wT[dt][:, et * 128:(et + 1) * 128],
                            xT[dt][:, rc * 512:(rc + 1) * 512],
                            start=(dt == 0), stop=(dt == 7))
                    cp(ci, dst[:, rc * 512:(rc + 1) * 512], pq[:]); ci += 1

            load_wT(2048, 1024)       # v weights
            for rt in range(8):
                for ec in range(2):
                    pv = psA.tile([128, 512], f32, tag="pqkv")
                    for dt in range(8):
                        nc.tensor.matmul(
                            pv[:],
                            xT[dt][:, rt * 128:(rt + 1) * 128],
                            wT[dt][:, ec * 512:(ec + 1) * 512],
                            start=(dt == 0), stop=(dt == 7))
                    cp(ci, v[rt][:, ec * 512:(ec + 1) * 512], pv[:]); ci += 1

        if DBG:
            nc.sync.dma_start(out=dbg_q[:, :], in_=qT[0][:, :].bitcast(f32))
        # ---------------- phase B: attention ----------------
        with tc.tile_pool(name="phB", bufs=1) as phB, \
             tc.tile_pool(name="attw", bufs=1) as attw, \
             tc.tile_pool(name="psD", bufs=2, space="PSUM") as psD, \
             tc.tile_pool(name="psAV", bufs=2, space="PSUM") as psAV, \
             tc.tile_pool(name="psM", bufs=1, space="PSUM") as psM:
            for it in range(4):
                its = slice(it * 128, (it + 1) * 128)
                E = phB.tile([128, H, NJ], bf16, tag="E")
                rs = phB.tile([128, H, 2], f32, tag="rs")
                rcp = phB.tile([128, H], f32, tag="rcp")
                for h in range(16):
                    et, po = h // 2, (h % 2) * 64
                    for jc in range(2):
                        js = slice(jc * 512, (jc + 1) * 512)
                        pd = psD.tile([128, 512], f32, tag="pdots")
                        nc.tensor.matmul(
                            pd[:],
                            qT[et][po:po + 64, its],
                            kT[et][po:po + 64, js],
                            start=True, stop=True)
                        nc.scalar.activation(
                            out=E[:, h, js], in_=pd[:],
                            func=Act.Exp, scale=SCALE,
                            accum_out=rs[:, h, jc:jc + 1])
                    nc.vector.tensor_add(rs[:, h, 0:1], rs[:, h, 0:1],
                                         rs[:, h, 1:2])
                nc.vector.reciprocal(rcp[:], rs[:, :, 0])
                for h in range(16):
                    nc.vector.tensor_scalar_mul(E[:, h, :], E[:, h, :],
                                                rcp[:, h:h + 1])

                if DBG and it == 0:
                    nc.gpsimd.dma_start(out=dbg_E[:, :, :], in_=E[:])
                # relayout: A[(i8,h), ig, j] <- E[ig*8+i8, h, j]
                A = phB.tile([128, 16, NJ], bf16, tag="A")
                for ig in range(16):
                    nc.sync.dma_start(
                        out=A[:, ig, :],
                        in_=E[ig * 8:(ig + 1) * 8, :, :])

                # head mix + LN (in-place into A)
                for ig in range(16):
                    for jc in range(2):
                        js = slice(jc * 512, (jc + 1) * 512)
                        pm = psM.tile([128, 512], f32, tag="pmix")
                        nc.tensor.matmul(pm[:], wblk[:], A[:, ig, js],
                                         start=True, stop=True)
                        M = phB.tile([128, 512], bf16, tag="M")
                        nc.vector.tensor_copy(M[:], pm[:])
                        if DBG and it == 0:
                            nc.gpsimd.dma_start(out=dbg_Am[:, ig, js], in_=M[:])
                        M2 = phB.tile([128, 512], bf16, tag="M2")
                        nc.vector.tensor_mul(M2[:], M[:], M[:])
                        s1 = psM.tile([8, 512], f32, tag="stat1")
                        nc.tensor.matmul(s1[:], sg[:], M[:],
                                         start=True, stop=True)
                        s2 = psM.tile([8, 512], f32, tag="stat2")
                        nc.tensor.matmul(s2[:], sg[:], M2[:],
                                         start=True, stop=True)
                        mu = phB.tile([8, 512], f32, tag="mu")
                        nc.scalar.mul(mu[:], s1[:], 1.0 / 16.0)
                        mu2 = phB.tile([8, 512], f32, tag="mu2")
                        nc.vector.tensor_mul(mu2[:], mu[:], mu[:])
                        var = phB.tile([8, 512], f32, tag="var")
                        nc.scalar.mul(var[:], s2[:], 1.0 / 16.0)
                        nc.vector.tensor_sub(var[:], var[:], mu2[:])
                        rstd = phB.tile([8, 512], f32, tag="rstd")
                        nc.scalar.activation(out=rstd[:], in_=var[:],
                                             func=Act.Sqrt,
                                             bias=eps_t[0:8, 0:1], scale=1.0)
                        nc.vector.reciprocal(rstd[:], rstd[:])
                        mub = phB.tile([128, 512], f32, tag="mub")
                        rstdb = phB.tile([128, 512], f32, tag="rstdb")
                        for dst, src_t in ((mub, mu), (rstdb, rstd)):
                            sap = src_t[:, :]
                            nc.sync.dma_start(
                                out=dst[:, :],
                                in_=AP(tensor=sap.tensor, offset=sap.offset,
                                       ap=[sap.ap[0], [0, 16], sap.ap[1]]))
                        nc.vector.tensor_sub(M[:], M[:], mub[:])
                        nc.vector.tensor_mul(M[:], M[:], rstdb[:])
                        nc.scalar.activation(out=A[:, ig, js], in_=M[:],
                                             func=Act.Identity,
                                             bias=lnb_t[:, 0:1],
                                             scale=lng_t[:, 0:1])

                if DBG and it == 0:
                    nc.gpsimd.dma_start(out=dbg_A[:, :, :], in_=A[:])
                # AV: transpose all A blocks first, then per-head
                # sequential PSUM chains (one start/stop pair at a time
                # per bank region), copying each head-pair out before the
                # next chain re-marks the zero region.
                atts = []
                ci2 = 0
                for jt in range(8):
                    att = attw.tile([128, 16, 8, 16], bf16, tag=f"att{jt}",
                                    name=f"att{jt}")
                    atts.append(att)
                    for ig in range(16):
                        pt = psM.tile([128, 128], bf16, tag="ptb")
                        nc.tensor.transpose(
                            pt[:], A[:, ig, jt * 128:(jt + 1) * 128], identb[:])
                        cp(ci2, att[:, ig, :, :].rearrange("p a b -> p (a b)"),
                           pt[:])
                        ci2 += 1
                for et in range(8):
                    av = psAV.tile([128, 128], f32, tag="av", name="av")
                    for hh in range(2):
                        h = 2 * et + hh
                        for jt in range(8):
                            nc.tensor.matmul(
                                av[hh * 64:(hh + 1) * 64, :],
                                v[jt][:, h * 64:(h + 1) * 64],
                                atts[jt][:, :, :, h],
                                start=(jt == 0), stop=(jt == 7),
                                skip_group_check=True)
                    cp(et, outT[et][:, its], av[:, :])

        # ---------------- phase C: output projection ----------------
        with tc.tile_pool(name="phC", bufs=1) as phC, \
             tc.tile_pool(name="tmpC", bufs=2) as tmpC, \
             tc.tile_pool(name="psC", bufs=2, space="PSUM") as psC:
            woT = [phC.tile([128, DIM], f32r, tag=f"woT{t}", name=f"woT{t}") for t in range(8)]
            ci = 0
            for et in range(8):
                wrow = tmpC.tile([128, DIM], f32, tag="row")
                nc.sync.dma_start(out=wrow[:],
                                  in_=w_out[et * 128:(et + 1) * 128, :])
                for dt in range(8):
                    pt = psC.tile([128, 128], f32, tag="ptr")
                    nc.tensor.transpose(pt[:], wrow[:, dt * 128:(dt + 1) * 128],
                                        ident[:])
                    cp(ci, woT[dt][:, et * 128:(et + 1) * 128], pt[:]); ci += 1
            for it in range(4):
                for mc in range(2):
                    pf = psC.tile([128, 512], f32, tag="pfin")
                    for et in range(8):
                        nc.tensor.matmul(
                            pf[:],
                            outT[et][:, it * 128:(it + 1) * 128],
                            woT[et][:, mc * 512:(mc + 1) * 512],
                            start=(et == 0), stop=(et == 7))
                    ob = tmpC.tile([128, 512], f32, tag="ob")
                    nc.vector.tensor_add(ob[:], pf[:],
                                         bb[:, mc * 512:(mc + 1) * 512])
                    nc.sync.dma_start(
                        out=out[it * 128:(it + 1) * 128,
                                mc * 512:(mc + 1) * 512],
                        in_=ob[:])


def _get_nc():
    if "nc" not in _CACHE:
        import concourse.bass as bass
        import concourse.mybir as mybir
        import concourse.tile as tile
        from concourse import bacc
        nc = bacc.Bacc("TRN2", target_bir_lowering=False, debug=False, num_devices=NCORES)
        with tile.TileContext(nc) as tc:
            _body(nc, tc, bass, mybir)
        nc.finalize()
        _CACHE["nc"] = nc
    return _CACHE["nc"]


def kernel(x, w_qkv, w_re, ln_g, ln_b, w_out, b_out, _trace=False):
    from concourse.bass_utils import run_bass_kernel_spmd
    nc = _get_nc()
    x = np.asarray(x, dtype=np.float32)
    in_maps = []
    for c in range(NCORES):
        ib, ih = c // 2, c % 2
        own = x[ib, ih * NI:(ih + 1) * NI]
        oth = x[ib, (1 - ih) * NI:(2 - ih) * NI]
        in_maps.append({
            "xkv": np.ascontiguousarray(np.concatenate([own, oth], axis=0)),
            "w_qkv": np.asarray(w_qkv, np.float32),
            "w_re": np.asarray(w_re, np.float32),
            "ln_g": np.asarray(ln_g, np.float32),
            "ln_b": np.asarray(ln_b, np.float32),
            "w_out": np.asarray(w_out, np.float32),
            "b_out": np.asarray(b_out, np.float32),
        })
    res = run_bass_kernel_spmd(nc, in_maps, list(range(NCORES)), trace=_trace)
    outp = np.empty((B, N, DIM), np.float32)
    for c in range(NCORES):
        ib, ih = c // 2, c % 2
        outp[ib, ih * NI:(ih + 1) * NI] = res.results[c]["out"]
    if _trace:
        return outp, res
    return outp


# revision 22
# speedup vs baseline: 1.1440x; 1.0417x over previous
"""DeepViT re-attention block on 8 TRN2 NeuronCores.

Sharding: core c -> batch ib=c//2, query-row half ih=c%2 (512 rows).
Each core computes k/v for its full batch (1024 rows) redundantly ->
zero collectives.  kv row order is core-local (own rows first), which
is fine: attention contracts over j order-invariantly.

Per-core pipeline (matmuls fp32r or bf16, PSUM accum f32):
  A. PE-transpose w_qkv -> wT and x -> xT; qkv projections:
     qT[e,i] (own rows), kT[e,j], v[j,e] (bf16, natural layout).
  B. per i-tile(128): dots = qT.T@kT (fp32r); exp on ACT (scale=1/8,
     accum_out = softmax denom); normalize (DVE); DMA-relayout
     [i,(h,j)] -> [(i8,h16),(ig,j)]; head-mix = block-diag(w_re^T)
     matmul; LN-over-h: ones-matmul stats + partition_broadcast +
     DVE/ACT apply (in-place); PE-transpose -> [j,(i8,h)]; AV matmul.
  C. out = outT.T @ w_outT + b_out -> DRAM.
"""

import numpy as np

B, N, DIM = 4, 1024, 1024
H, DH = 16, 64
SCALE = DH ** -0.5
EPS = 1e-5
NI = 512
NJ = 1024
NCORES = 8

_CACHE = {}


def _body(nc, tc, bass, mybir):
    f32 = mybir.dt.float32
    f32r = mybir.dt.float32r
    bf16 = mybir.dt.bfloat16
    Act = mybir.ActivationFunctionType
    Alu = mybir.AluOpType
    AP = bass.AP

    xkv = nc.declare_dram_parameter("xkv", [NJ, DIM], f32, isOutput=False)
    w_qkv = nc.declare_dram_parameter("w_qkv", [3 * DIM, DIM], f32, isOutput=False)
    w_re = nc.declare_dram_parameter("w_re", [H, H], f32, isOutput=False)
    ln_g = nc.declare_dram_parameter("ln_g", [H], f32, isOutput=False)
    ln_b = nc.declare_dram_parameter("ln_b", [H], f32, isOutput=False)
    w_out = nc.declare_dram_parameter("w_out", [DIM, DIM], f32, isOutput=False)
    b_out = nc.declare_dram_parameter("b_out", [DIM], f32, isOutput=False)
    out = nc.declare_dram_parameter("out", [NI, DIM], f32, isOutput=True)
    import os
    DBG = bool(os.environ.get("KERNEL_DEBUG_DUMPS"))
    if DBG:
        dbg_E = nc.declare_dram_parameter("dbg_E", [128, H, NJ], f32, isOutput=True)
        dbg_Am = nc.declare_dram_parameter("dbg_Am", [128, 16, NJ], f32, isOutput=True)
        dbg_A = nc.declare_dram_parameter("dbg_A", [128, 16, NJ], f32, isOutput=True)
        dbg_q = nc.declare_dram_parameter("dbg_q", [128, NI], f32, isOutput=True)

    def cp(i, dst, src):
        # alternate copies between DVE and ACT to balance engine load
        if i % 2 == 0:
            nc.vector.tensor_copy(dst, src)
        else:
            nc.scalar.copy(dst, src)

    with tc.tile_pool(name="const", bufs=1) as const, \
         tc.tile_pool(name="big", bufs=1) as big:
        # ---------------- constants ----------------
        ident = const.tile([128, 128], f32)
        nc.gpsimd.memset(ident[:], 1.0)
        nc.gpsimd.affine_select(out=ident[:], in_=ident[:],
                                compare_op=Alu.is_ge, fill=0.0, base=0,
                                pattern=[[-1, 128]], channel_multiplier=1)
        nc.gpsimd.affine_select(out=ident[:], in_=ident[:],
                                compare_op=Alu.is_ge, fill=0.0, base=0,
                                pattern=[[1, 128]], channel_multiplier=-1)
        identb = const.tile([128, 128], bf16)
        nc.vector.tensor_copy(identb[:], ident[:])

        wret_f = const.tile([16, 16], f32)
        nc.sync.dma_start(out=wret_f[:], in_=w_re.rearrange("g h -> h g"))
        wret = const.tile([16, 16], bf16)
        nc.vector.tensor_copy(wret[:], wret_f[:])
        wblk = const.tile([128, 128], bf16)
        nc.vector.memset(wblk[:], 0.0)
        for i8 in range(8):
            nc.sync.dma_start(
                out=wblk[i8 * 16:(i8 + 1) * 16, i8 * 16:(i8 + 1) * 16],
                in_=wret[:, :])

        # Sg[(i8,g), i8'] = 1 if i8 == i8' else 0   (bf16, [128, 8])
        sg = const.tile([128, 8], bf16)
        nc.gpsimd.memset(sg[:], 1.0)
        nc.gpsimd.affine_select(out=sg[:], in_=sg[:], compare_op=Alu.is_ge,
                                fill=0.0, base=0, pattern=[[-16, 8]],
                                channel_multiplier=1)
        nc.gpsimd.affine_select(out=sg[:], in_=sg[:], compare_op=Alu.is_ge,
                                fill=0.0, base=15, pattern=[[16, 8]],
                                channel_multiplier=-1)

        # ln_g/ln_b replicated to [(i8,h), 1]
        lng_t = const.tile([128, 1], f32)
        lnb_t = const.tile([128, 1], f32)
        nc.sync.dma_start(
            out=lng_t[:, 0:1],
            in_=AP(tensor=ln_g, offset=0, ap=[[0, 8], [1, 16], [0, 1]]))
        nc.sync.dma_start(
            out=lnb_t[:, 0:1],
            in_=AP(tensor=ln_b, offset=0, ap=[[0, 8], [1, 16], [0, 1]]))

        eps_t = const.tile([128, 1], f32)
        nc.vector.memset(eps_t[:], EPS)
        bb = const.tile([128, DIM], f32)
        nc.sync.dma_start(out=bb[:],
                          in_=AP(tensor=b_out, offset=0,
                                 ap=[[0, 128], [1, DIM]]))

        # persistent activations
        qT = [big.tile([128, NI], f32r, tag=f"qT{t}", name=f"qT{t}") for t in range(8)]
        kT = [big.tile([128, NJ], f32r, tag=f"kT{t}", name=f"kT{t}") for t in range(8)]
        v = [big.tile([128, DIM], bf16, tag=f"v{t}", name=f"v{t}") for t in range(8)]
        outT = [big.tile([128, NI], f32r, tag=f"oT{t}", name=f"oT{t}") for t in range(8)]

        # ---------------- phase A: transposes + qkv ----------------
        with tc.tile_pool(name="phA", bufs=1) as phA, \
             tc.tile_pool(name="tmpA", bufs=2) as tmpA, \
             tc.tile_pool(name="psA", bufs=3, space="PSUM") as psA:
            xT = [phA.tile([128, NJ], f32r, tag=f"xT{t}", name=f"xT{t}") for t in range(8)]
            ci = 0
            for rt in range(8):
                xrow = tmpA.tile([128, DIM], f32, tag="row")
                nc.sync.dma_start(out=xrow[:],
                                  in_=xkv[rt * 128:(rt + 1) * 128, :])
                for dt in range(8):
                    pt = psA.tile([128, 128], f32, tag="ptr")
                    nc.tensor.transpose(pt[:], xrow[:, dt * 128:(dt + 1) * 128],
                                        ident[:])
                    cp(ci, xT[dt][:, rt * 128:(rt + 1) * 128], pt[:]); ci += 1

            wT = [phA.tile([128, 2048], f32r, tag=f"wT{t}", name=f"wT{t}") for t in range(8)]

            def load_wT(e0, ne):
                nonlocal ci
                for et in range(ne // 128):
                    wrow = tmpA.tile([128, DIM], f32, tag="row")
                    nc.sync.dma_start(
                        out=wrow[:],
                        in_=w_qkv[e0 + et * 128:e0 + (et + 1) * 128, :])
                    for dt in range(8):
                        pt = psA.tile([128, 128], f32, tag="ptr")
                        nc.tensor.transpose(
                            pt[:], wrow[:, dt * 128:(dt + 1) * 128], ident[:])
                        cp(ci, wT[dt][:, et * 128:(et + 1) * 128], pt[:])
                        ci += 1

            load_wT(0, 2048)          # q + k weights
            for et in range(16):
                dst, cols = (qT[et], NI) if et < 8 else (kT[et - 8], NJ)
                for rc in range(cols // 512):
                    pq = psA.tile([128, 512], f32, tag="pqkv")
                    for dt in range(8):
                        nc.tensor.matmul(
                            pq[:],
                            wT[dt][:, et * 128:(et + 1) * 128],
                            xT[dt][:, rc * 512:(rc + 1) * 512],
                            start=(dt == 0), stop=(dt == 7))
                    cp(ci, dst[:, rc * 512:(rc + 1) * 512], pq[:]); ci += 1

            load_wT(2048, 1024)       # v weights
            for rt in range(8):
                for ec in range(2):
                    pv = psA.tile([128, 512], f32, tag="pqkv")
                    for dt in range(8):
                        nc.tensor.matmul(
                            pv[:],
                            xT[dt][:, rt * 128:(rt + 1) * 128],
                            wT[dt][:, ec * 512:(ec + 1) * 512],
                            start=(dt == 0), stop=(dt == 7))
                    cp(ci, v[rt][:, ec * 512:(ec + 1) * 512], pv[:]); ci += 1

        if DBG:
            nc.sync.dma_start(out=dbg_q[:, :], in_=qT[0][:, :].bitcast(f32))
        # ---------------- phase B: attention ----------------
        with tc.tile_pool(name="phB", bufs=1) as phB, \
             tc.tile_pool(name="attw", bufs=1) as attw, \
             tc.tile_pool(name="psD", bufs=2, space="PSUM") as psD, \
             tc.tile_pool(name="psAV", bufs=2, space="PSUM") as psAV, \
             tc.tile_pool(name="psM", bufs=1, space="PSUM") as psM:
            for it in range(4):
                its = slice(it * 128, (it + 1) * 128)
                E = phB.tile([128, H, NJ], bf16, tag="E")
                rs = phB.tile([128, H, 2], f32, tag="rs")
                rcp = phB.tile([128, H], f32, tag="rcp")
                for h in range(16):
                    et, po = h // 2, (h % 2) * 64
                    for jc in range(2):
                        js = slice(jc * 512, (jc + 1) * 512)
                        pd = psD.tile([128, 512], f32, tag="pdots")
                        nc.tensor.matmul(
                            pd[:],
                            qT[et][po:po + 64, its],
                            kT[et][po:po + 64, js],
                            start=True, stop=True)
                        nc.scalar.activation(
                            out=E[:, h, js], in_=pd[:],
                            func=Act.Exp, scale=SCALE,
                            accum_out=rs[:, h, jc:jc + 1])
                    nc.vector.tensor_add(rs[:, h, 0:1], rs[:, h, 0:1],
                                         rs[:, h, 1:2])
                nc.vector.reciprocal(rcp[:], rs[:, :, 0])
                for h in range(16):
                    nc.vector.tensor_scalar_mul(E[:, h, :], E[:, h, :],
                                                rcp[:, h:h + 1])

                if DBG and it == 0:
                    nc.gpsimd.dma_start(out=dbg_E[:, :, :], in_=E[:])
                # relayout: A[(i8,h), ig, j] <- E[ig*8+i8, h, j]
                A = phB.tile([128, 16, NJ], bf16, tag="A")
                for ig in range(16):
                    nc.sync.dma_start(
                        out=A[:, ig, :],
                        in_=E[ig * 8:(ig + 1) * 8, :, :])

                # head mix + LN (in-place into A)
                for ig in range(16):
                    for jc in range(2):
                        js = slice(jc * 512, (jc + 1) * 512)
                        pm = psM.tile([128, 512], f32, tag="pmix")
                        nc.tensor.matmul(pm[:], wblk[:], A[:, ig, js],
                                         start=True, stop=True)
                        M = phB.tile([128, 512], bf16, tag="M")
                        nc.vector.tensor_copy(M[:], pm[:])
                        if DBG and it == 0:
                            nc.gpsimd.dma_start(out=dbg_Am[:, ig, js], in_=M[:])
                        M2 = phB.tile([128, 512], bf16, tag="M2")
                        nc.vector.tensor_mul(M2[:], M[:], M[:])
                        st = psM.tile([128, 512], f32, tag="stat")
                        nc.tensor.matmul(st[0:8, :], sg[:], M[:],
                                         start=True, stop=True)
                        nc.tensor.matmul(st[64:72, :], sg[:], M2[:],
                                         start=True, stop=True)
                        mu = phB.tile([8, 512], f32, tag="mu")
                        nc.scalar.mul(mu[:], st[0:8, :], 1.0 / 16.0)
                        mu2 = phB.tile([8, 512], f32, tag="mu2")
                        nc.vector.tensor_mul(mu2[:], mu[:], mu[:])
                        var = phB.tile([8, 512], f32, tag="var")
                        nc.scalar.mul(var[:], st[64:72, :], 1.0 / 16.0)
                        nc.vector.tensor_sub(var[:], var[:], mu2[:])
                        rstd = phB.tile([8, 512], f32, tag="rstd")
                        nc.scalar.activation(out=rstd[:], in_=var[:],
                                             func=Act.Sqrt,
                                             bias=eps_t[0:8, 0:1], scale=1.0)
                        nc.vector.reciprocal(rstd[:], rstd[:])
                        mub = phB.tile([128, 512], f32, tag="mub")
                        rstdb = phB.tile([128, 512], f32, tag="rstdb")
                        for dst, src_t in ((mub, mu), (rstdb, rstd)):
                            sap = src_t[:, :]
                            nc.sync.dma_start(
                                out=dst[:, :],
                                in_=AP(tensor=sap.tensor, offset=sap.offset,
                                       ap=[sap.ap[0], [0, 16], sap.ap[1]]))
                        nc.vector.tensor_sub(M[:], M[:], mub[:])
                        nc.vector.tensor_mul(M[:], M[:], rstdb[:])
                        nc.scalar.activation(out=A[:, ig, js], in_=M[:],
                                             func=Act.Identity,
                                             bias=lnb_t[:, 0:1],
                                             scale=lng_t[:, 0:1])

                if DBG and it == 0:
                    nc.gpsimd.dma_start(out=dbg_A[:, :, :], in_=A[:])
                # AV: transpose all A blocks first, then per-head
                # sequential PSUM chains (one start/stop pair at a time
                # per bank region), copying each head-pair out before the
                # next chain re-marks the zero region.
                atts = []
                ci2 = 0
                for jt in range(8):
                    att = attw.tile([128, 16, 8, 16], bf16, tag=f"att{jt}",
                                    name=f"att{jt}")
                    atts.append(att)
                    for ig in range(16):
                        pt = psD.tile([128, 128], bf16, tag="ptb")
                        nc.tensor.transpose(
                            pt[:], A[:, ig, jt * 128:(jt + 1) * 128], identb[:])
                        cp(ci2, att[:, ig, :, :].rearrange("p a b -> p (a b)"),
                           pt[:])
                        ci2 += 1
                for et in range(8):
                    av = psAV.tile([128, 128], f32, tag="av", name="av")
                    for hh in range(2):
                        h = 2 * et + hh
                        for jt in range(8):
                            nc.tensor.matmul(
                                av[hh * 64:(hh + 1) * 64, :],
                                v[jt][:, h * 64:(h + 1) * 64],
                                atts[jt][:, :, :, h],
                                start=(jt == 0), stop=(jt == 7),
                                skip_group_check=True)
                    cp(et, outT[et][:, its], av[:, :])

        # ---------------- phase C: output projection ----------------
        with tc.tile_pool(name="phC", bufs=1) as phC, \
             tc.tile_pool(name="tmpC", bufs=2) as tmpC, \
             tc.tile_pool(name="psC", bufs=2, space="PSUM") as psC:
            woT = [phC.tile([128, DIM], f32r, tag=f"woT{t}", name=f"woT{t}") for t in range(8)]
            ci = 0
            for et in range(8):
                wrow = tmpC.tile([128, DIM], f32, tag="row")
                nc.sync.dma_start(out=wrow[:],
                                  in_=w_out[et * 128:(et + 1) * 128, :])
                for dt in range(8):
                    pt = psC.tile([128, 128], f32, tag="ptr")
                    nc.tensor.transpose(pt[:], wrow[:, dt * 128:(dt + 1) * 128],
                                        ident[:])
                    cp(ci, woT[dt][:, et * 128:(et + 1) * 128], pt[:]); ci += 1
            for it in range(4):
                for mc in range(2):
                    pf = psC.tile([128, 512], f32, tag="pfin")
                    for et in range(8):
                        nc.tensor.matmul(
                            pf[:],
                            outT[et][:, it * 128:(it + 1) * 128],
                            woT[et][:, mc * 512:(mc + 1) * 512],
                            start=(et == 0), stop=(et == 7))
                    ob = tmpC.tile([128, 512], f32, tag="ob")
                    nc.vector.tensor_add(ob[:], pf[:],
                                         bb[:, mc * 512:(mc + 1) * 512])
                    nc.sync.dma_start(
                        out=out[it * 128:(it + 1) * 128,
                                mc * 512:(mc + 1) * 512],
                        in_=ob[:])


def _get_nc():
    if "nc" not in _CACHE:
        import concourse.bass as bass
        import concourse.mybir as mybir
        import concourse.tile as tile
        from concourse import bacc
        nc = bacc.Bacc("TRN2", target_bir_lowering=False, debug=False, num_devices=NCORES)
        with tile.TileContext(nc) as tc:
            _body(nc, tc, bass, mybir)
        nc.finalize()
        _CACHE["nc"] = nc
    return _CACHE["nc"]


def kernel(x, w_qkv, w_re, ln_g, ln_b, w_out, b_out, _trace=False):
    from concourse.bass_utils import run_bass_kernel_spmd
    nc = _get_nc()
    x = np.asarray(x, dtype=np.float32)
    in_maps = []
    for c in range(NCORES):
        ib, ih = c // 2, c % 2
        own = x[ib, ih * NI:(ih + 1) * NI]
        oth = x[ib, (1 - ih) * NI:(2 - ih) * NI]
        in_maps.append({
            "xkv": np.ascontiguousarray(np.concatenate([own, oth], axis=0)),
            "w_qkv": np.asarray(w_qkv, np.float32),
            "w_re": np.asarray(w_re, np.float32),
            "ln_g": np.asarray(ln_g, np.float32),
            "ln_b": np.asarray(ln_b, np.float32),
            "w_out": np.asarray(w_out, np.float32),
            "b_out": np.asarray(b_out, np.float32),
        })
    res = run_bass_kernel_spmd(nc, in_maps, list(range(NCORES)), trace=_trace)
    outp = np.empty((B, N, DIM), np.float32)
    for c in range(NCORES):
        ib, ih = c // 2, c % 2
        outp[ib, ih * NI:(ih + 1) * NI] = res.results[c]["out"]
    if _trace:
        return outp, res
    return outp
